# revision 29
# baseline (speedup 1.0000x reference)
"""Trainium2 Bass kernel for nn_DSC_86071144612259.

The reference network collapses to a single linear contraction

    u[b, c] = sum_{d<128} sum_{p} W[d, p, c] * y_rev[b, d, p]

where W [128, P, MC] is assembled from the small parameter tensors
(M0 / M_tilde / M_0l / M_big / sigma / lambda_e / phi / phi_tilde).
History slot 128 (the last of the 129) has zero coefficient, so only
128 delays contribute.  W is tiny (128*32*16 fp), so it is computed
exactly on the host in float64; the 270 MB y_rev contraction is the
real work and is purely memory bound.

Sharding: pure data parallel over the batch axis across the 8 cores
(2048 rows each).  W is replicated.

Precision: y is cast on the host to fp8 E3M4 (TRN FP8_EXP3: 4
mantissa bits, range +-15.5 — y absmax is 5.42), which HALVES the
HBM traffic vs fp16; measured absmax-relative output error 1.31e-2
against the fp64 oracle, under the 2e-2 gate.  W/32 (absmax 15.07)
is split into W_hi = e3m4(W/32) plus W_lo = e3m4(W/32 - W_hi); the
two are stacked as 32 stationary columns of a single matmul, so the
residual correction costs no extra PE instructions.  The host adds
the hi/lo output stripes and multiplies by 32.

Layout/streaming (per core): the host packs W's swizzled bytes and
the fp8 y-shard [K=4096, BS=2048] into ONE dram tensor yw
[128, 1024 + 32*2048]: partition row p holds [w_p | chunk0_p | ... |
chunk31_p], every DMA fully contiguous on both sides.  The two HWDGE
rings (sync / scalar queues) alternate 4-chunk packs — always all
128 partitions (subsets run at half engine bandwidth) with 8 KB
descriptors, so per-ring descriptor generation (~22 ns/descriptor)
outruns the ~400 GB/s the 16 SDMA engines drain — and finish with
one single-chunk DMA each so the tensor engine's final chase unit is
small.  The 4 batch-chunks of each k-chunk run in disjoint 32-column
PE groups (tile_position), striping u^T across PSUM partitions
32*bc + c (hi) / 32*bc + 16 + c (lo); the tail is two half-width
PSUM->SBUF casts on the vector engine and one 64-partition store per
ring with no completion wait (the NEFF epilogue drains the rings,
saving the receipt round-trip).
"""

import numpy as np

B = 16384      # batch
L = 129        # history length of y_rev
P = 32         # observation dim
MC = 16        # control dim (output)
H = 24         # spectral dim
M = 64         # filter length
NCORES = 8
BS = B // NCORES           # 2048 batch rows per core
KD = 128                   # delays with nonzero weight
K = KD * P                 # 4096 contraction length
NKC = K // 128             # 32 k-chunks of 128 partitions
NFREE = 512                # matmul moving free dim (one fp32 PSUM bank)
NB = BS // NFREE           # 4 batch chunks per core
E3M4_MAX = 15.5            # TRN FP8_EXP3 max normal
WCOLS = NKC * 2 * MC       # 1024 stationary columns (hi|lo per chunk)

_CACHE = {}


def _build_w(M0, M_tilde, M_0l, M_big, sigma, lambda_e, phi, phi_tilde):
    """Collapse the parameter tensors into W [KD, MC, P] (float64).

    Mirrors reference.py exactly:
      term1: delay 0,      M0
      term2: delays 1..64, sum_i lambda_i^0.25 phi_tilde[j-1,i] M_tilde[i]
      term3: delays 0..63, sum_l sigma_l^0.25  phi[k,l]         M_0l[l]
      term4: delays 1..127 via conv(phi_tilde[:,i], phi[:,l]) and M_big
    """
    f8 = np.float64
    M0 = M0.astype(f8)
    M_tilde = M_tilde.astype(f8)
    M_0l = M_0l.astype(f8)
    M_big = M_big.astype(f8)
    sigma = sigma.astype(f8)
    lambda_e = lambda_e.astype(f8)
    phi = phi.astype(f8)
    phi_tilde = phi_tilde.astype(f8)

    W = np.zeros((KD, MC, P), dtype=f8)
    W[0] += M0
    pt = phi_tilde * (lambda_e ** 0.25)[None, :]
    W[1:M + 1] += np.einsum("ji,icp->jcp", pt, M_tilde)
    ps = phi * (sigma ** 0.25)[None, :]
    W[0:M] += np.einsum("kl,lcp->kcp", ps, M_0l)
    W4 = np.empty((H, H, 2 * M - 1), dtype=f8)
    for i in range(H):
        for l in range(H):
            W4[i, l] = np.convolve(phi_tilde[:, i], phi[:, l])
    scale = (lambda_e[:, None] * sigma[None, :]) ** 0.25
    W[1:2 * M] += np.einsum("ild,ilcp->dcp", W4 * scale[:, :, None], M_big)
    return W


# y-DMA schedule: (ring, first chunk, n chunks) packs, alternating the
# two HWDGE rings.  Every DMA emits 128 descriptors (one per partition;
# partition-subset DMAs run at half engine bandwidth, so always use all
# 128) of nch*2KB each, generated FIFO per ring at ~22 ns/descriptor:
# packs of >=2 chunks keep per-ring generation bandwidth above what the
# 16 SDMA engines drain (~400 GB/s aggregate), and the trailing
# [28-30]/[31] split keeps the tensor engine's final chase unit small.
_SCHED = [
    ("sync", 0, 2), ("scalar", 2, 2),
    ("sync", 4, 4), ("scalar", 8, 4),
    ("sync", 12, 4), ("scalar", 16, 4),
    ("sync", 20, 4), ("scalar", 24, 4),
    ("sync", 28, 1), ("scalar", 29, 1),
    ("sync", 30, 1), ("scalar", 31, 1),
]


def _get_nc():
    """Build the per-core Bass program (cached)."""
    if "nc" in _CACHE:
        return _CACHE["nc"]
    import concourse.bass as bass
    import concourse.mybir as mybir

    f8 = mybir.dt.float8e3
    nc = bass.Bass("TRN2", target_bir_lowering=False, enable_partition_id=False)
    yw = nc.dram_tensor("yw", [128, WCOLS + NKC * BS], f8, kind="ExternalInput")
    ut = nc.dram_tensor("ut", [128, NFREE], mybir.dt.float16, kind="ExternalOutput")

    # One SBUF tensor, same column layout as yw: w then the 32 y chunks.
    yw_sb = nc.alloc_sbuf_tensor("yw_sb", [128, WCOLS + NKC * BS], f8)
    u_sb = nc.alloc_sbuf_tensor("u_sb", [128, NFREE], mybir.dt.float16)
    ps = nc.alloc_psum_tensor("ps", [128, NFREE], mybir.dt.float32)
    sem_y = [nc.alloc_semaphore(f"sem_y{g}") for g in range(len(_SCHED))]
    pe_done = nc.alloc_semaphore("pe_done")
    ve_done = nc.alloc_semaphore("ve_done")
    odma = nc.alloc_semaphore("odma")

    def col0(c):
        # first yw column of chunk c (the w block precedes chunk 0)
        return WCOLS + c * BS

    def issue(q, ring):
        for g, (r, first, nch) in enumerate(_SCHED):
            if r != ring:
                continue
            lo = col0(first) if first else 0          # first pack includes w
            hi = col0(first + nch)
            q.dma_start(
                out=yw_sb[:, lo:hi], in_=yw[:, lo:hi]
            ).then_inc(sem_y[g], 16)

    with nc.Block() as block:

        @block.sync
        def _(sync):
            issue(sync, "sync")
            # store u^T — split by partition half across the rings; no
            # explicit completion wait: the NEFF epilogue drains the rings.
            sync.wait_ge(ve_done, 2)
            sync.dma_start(out=ut[:64, :], in_=u_sb[:64, :]).then_inc(odma, 16)

        @block.scalar
        def _(scalar):
            issue(scalar, "scalar")
            scalar.wait_ge(ve_done, 2)
            scalar.dma_start(out=ut[64:, :], in_=u_sb[64:, :]).then_inc(odma, 16)

        @block.tensor
        def _(tensor):
            for g, (r, first, nch) in enumerate(_SCHED):
                tensor.wait_ge(sem_y[g], 16)
                last_group = g == len(_SCHED) - 1
                for ki in range(first, first + nch):
                    if last_group and ki == first + nch - 1:
                        break
                    for bc in range(NB):
                        tensor.matmul(
                            ps[32 * bc:32 * (bc + 1), :],
                            yw_sb[:, 32 * ki:32 * (ki + 1)],
                            yw_sb[:, col0(ki) + bc * NFREE:col0(ki) + (bc + 1) * NFREE],
                            start=(ki == 0),
                            stop=False,
                            tile_position=(0, 32 * bc),
                        )
            # Last chunk in two N=256 halves so the copy of the first half
            # overlaps the second half's matmuls.
            ki = NKC - 1
            for half in range(2):
                lo, hi = half * NFREE // 2, (half + 1) * NFREE // 2
                for bc in range(NB):
                    mm = tensor.matmul(
                        ps[32 * bc:32 * (bc + 1), lo:hi],
                        yw_sb[:, 32 * ki:32 * (ki + 1)],
                        yw_sb[:, col0(ki) + bc * NFREE + lo:col0(ki) + bc * NFREE + hi],
                        start=False,
                        stop=True,
                        tile_position=(0, 32 * bc),
                    )
                    if bc == NB - 1:
                        mm.then_inc(pe_done, 1)

        @block.vector
        def _(vector):
            for half in range(2):
                lo, hi = half * NFREE // 2, (half + 1) * NFREE // 2
                vector.wait_ge(pe_done, half + 1)
                vector.tensor_copy(
                    out=u_sb[:, lo:hi], in_=ps[:, lo:hi]
                ).then_inc(ve_done, 1)

    _CACHE["nc"] = nc
    return nc


def _ensure_ntff_hook():
    """bass_utils hard-imports antenv.axon_hooks when BASS_TRACE is set;
    this container's trimmed antenv lacks it.  Register a working stub
    built from trn_agent_boot's ctypes NTFF driver (or a None hook,
    which bass_utils degrades gracefully on)."""
    import importlib.util
    import sys
    import types

    if "antenv.axon_hooks" in sys.modules:
        return
    try:
        if importlib.util.find_spec("antenv.axon_hooks") is not None:
            return
    except (ImportError, ValueError):
        pass
    try:
        from trn_agent_boot.trn_boot import _ntff_profile_via_ctypes

        hook = _ntff_profile_via_ctypes("/opt/axon/libaxon_pjrt.so")
    except Exception:
        hook = None
    mod = types.ModuleType("antenv.axon_hooks")
    mod.get_axon_ntff_profile_hook = lambda: hook
    sys.modules["antenv.axon_hooks"] = mod


def kernel(y_rev, M0, M_tilde, M_0l, M_big, sigma, lambda_e, phi, phi_tilde):
    _ensure_ntff_hook()
    import ml_dtypes
    from concourse.bass_utils import run_bass_kernel_spmd

    e3m4 = ml_dtypes.float8_e3m4
    y_rev = np.asarray(y_rev)
    W = _build_w(*[np.asarray(a) for a in (
        M0, M_tilde, M_0l, M_big, sigma, lambda_e, phi, phi_tilde)])
    # W_flat[k, c] with k = d*P + p; scaled by a power of 2 (32 for the
    # seed-0 inputs: absmax 482 -> 15.07) to fit e3m4's +-15.5, then
    # split hi + lo residual.
    WS = float(2.0 ** np.ceil(np.log2(np.abs(W).max() / E3M4_MAX)))
    Wf = (W.transpose(0, 2, 1).reshape(K, MC) / WS).astype(np.float32)
    W1 = Wf.astype(e3m4)
    W2 = (Wf - W1.astype(np.float32)).astype(e3m4)
    # Stack: chunk ki -> stationary columns [32ki, 32ki+16) hi, +16 lo.
    Wd = np.empty((NKC, 128, 2 * MC), dtype=e3m4)
    Wd[:, :, :MC] = W1.reshape(NKC, 128, MC)
    Wd[:, :, MC:] = W2.reshape(NKC, 128, MC)
    Wd = np.ascontiguousarray(Wd.transpose(1, 0, 2).reshape(128, WCOLS))

    in_maps = []
    for s in range(NCORES):
        blk = y_rev[s * BS:(s + 1) * BS, :KD, :].astype(e3m4)
        ytp = blk.reshape(BS, K).T                       # [K, BS]
        # partition row p: [w_p | chunk0 row p | ... | chunk31 row p]
        ywp = np.empty((128, WCOLS + NKC * BS), dtype=e3m4)
        ywp[:, :WCOLS] = Wd
        ywp[:, WCOLS:] = (
            ytp.reshape(NKC, 128, BS).transpose(1, 0, 2).reshape(128, NKC * BS)
        )
        in_maps.append({"yw": ywp})

    res = run_bass_kernel_spmd(_get_nc(), in_maps, list(range(NCORES)))
    _CACHE["last_result"] = res

    out = np.empty((B, MC), dtype=np.float32)
    for s in range(NCORES):
        # ut[32*bc + c, j] hi and ut[32*bc + 16 + c, j] lo of
        # u^T[c, bc*512 + j] / WS, raw fp32 PSUM.
        stripes = res.results[s]["ut"].reshape(NB, 2, MC, NFREE).astype(np.float32)
        u = (stripes[:, 0] + stripes[:, 1]) * WS          # [NB, MC, NFREE]
        out[s * BS:(s + 1) * BS, :] = u.transpose(0, 2, 1).reshape(BS, MC)
    return out
